# revision 1
# baseline (speedup 1.0000x reference)
"""DGCNN-style GNN (2x dynamic-kNN EdgeConv + global pool + MLP head) on 8 Trainium2
NeuronCores, data-parallel over the 512 graphs (64 graphs per core).

Self-contained: hardcodes all shapes; host side only does layout prep (transpose /
tiling / block-diag packing of weights) and sharding.
"""

import sys

for _p in ("/opt/trn_rl_repo",):
    if _p not in sys.path:
        sys.path.append(_p)

from contextlib import ExitStack

import numpy as np

import concourse.bass as bass
import concourse.tile as tile
from concourse import bacc, mybir
from concourse.bass_utils import run_bass_kernel_spmd

F32 = mybir.dt.float32
U16 = mybir.dt.uint16
I16 = mybir.dt.int16
AF = mybir.ActivationFunctionType
ALU = mybir.AluOpType
AX = mybir.AxisListType

B, N, K = 512, 256, 20
NCORES = 8
GPC = B // NCORES  # graphs per core = 64
NEG = -1.0e30


# ---------------------------------------------------------------------------
# Bass program builder (identical program for every core; all data via inputs)
# ---------------------------------------------------------------------------

def build_program(sets: int = 8):
    """Build the per-core Bass program. `sets` = number of 8-graph sets (8 => 64
    graphs). Returns the compiled Bacc object."""
    G = 8 * sets
    nc = bacc.Bacc("TRN2", target_bir_lowering=False, debug=False)

    def din(name, shape, dtype=F32):
        return nc.declare_dram_parameter(name, list(shape), dtype, isOutput=False)

    # -------------------- DRAM parameters --------------------
    xtf_d = din("xtf", [128, 256])          # [2g+f, j] = x[256g+j, f] (g<64)
    a4_d = din("a4all", [4, 16384])         # rows: x0, x1, ones, zeros
    b4_d = din("b4all", [4, 16384])         # rows: x0, x1, zeros(->-sq/2), ones
    cwrep_d = din("cwrep", [2, 128])        # tile8(c1_w0[:2]-c1_w0[2:4])
    bwrep_d = din("bwrep", [2, 128])        # tile8(c1_w0[2:4])
    b0rep_d = din("b0rep", [128, 1])        # tile8(c1_b0)
    w1bd_d = din("w1bd", [128, 128])        # blkdiag8(c1_w1)
    b1rep_d = din("b1rep", [128, 1])
    w2bd_d = din("w2bd", [128, 128])        # blkdiag8(c1_w2)
    b2rep_d = din("b2rep", [128, 1])
    bdgf_d = din("bdgf", [128, 64])         # [2g+f, g] = -0.5
    nh16_d = din("neghalf16", [16, 1])      # -0.5
    ones_d = din("onesrow", [1, 2048])
    zeros_d = din("zerorow", [1, 2048])
    iota_d = din("iotaidx", [128, 40], I16)  # [:, 20t:20t+20] = 128t+p (idx of self)
    qarep_d = din("wb2repA", [16, 128])     # tile8(c2_w0[16:32, :16])
    qbrep_d = din("wb2repB", [16, 128])     # tile8(c2_w0[16:32, 16:])
    wd2aug_d = din("wd2aug", [18, 32])      # [c2_w0[:16]-c2_w0[16:32]; 0; c2_b0]
    w1l_d = din("w1l", [48, 128])           # lin1_w
    b1l_d = din("b1l", [128, 1])            # lin1_b
    mw0_d = din("mw0", [128, 64])
    mb0_d = din("mb0", [64, 1])
    mw1_d = din("mw1", [64, 64])
    mb1_d = din("mb1", [64, 1])
    mw2_d = din("mw2", [64, 1])
    mb2_d = din("mb2", [1, 1])
    out_d = nc.declare_dram_parameter("out", [1, G], F32, isOutput=True)

    with tile.TileContext(nc) as tc, ExitStack() as ctx:
        P = lambda **kw: ctx.enter_context(tc.tile_pool(**kw))
        wp = P(name="weights", bufs=1)

        def load(dram, shape, dtype=F32):
            t = wp.tile(list(shape), dtype, tag=dram.name)
            nc.sync.dma_start(t[:], dram.ap())
            return t

        xtf = load(xtf_d, [128, 256])
        cwrep = load(cwrep_d, [2, 128])
        bwrep = load(bwrep_d, [2, 128])
        b0rep = load(b0rep_d, [128, 1])
        w1bd = load(w1bd_d, [128, 128])
        b1rep = load(b1rep_d, [128, 1])
        w2bd = load(w2bd_d, [128, 128])
        b2rep = load(b2rep_d, [128, 1])
        bdgf = load(bdgf_d, [128, 64])
        nh16 = load(nh16_d, [16, 1])
        onesr = load(ones_d, [1, 2048])
        zeror = load(zeros_d, [1, 2048])
        iota = load(iota_d, [128, 40], I16)
        qarep_w = load(qarep_d, [16, 128])
        qbrep_w = load(qbrep_d, [16, 128])
        wd2aug = load(wd2aug_d, [18, 32])
        w1l = load(w1l_d, [48, 128])
        b1l = load(b1l_d, [128, 1])
        mw0 = load(mw0_d, [128, 64])
        mb0 = load(mb0_d, [64, 1])
        mw1 = load(mw1_d, [64, 64])
        mb1 = load(mb1_d, [64, 1])
        mw2 = load(mw2_d, [64, 1])
        mb2 = load(mb2_d, [1, 1])

        # persistent core-level tensors
        pooledT = wp.tile([128, G], F32)

        # pools
        pl_sc = P(name="scps", bufs=2, space="PSUM")      # [128,256] score psums
        pl_tb = P(name="tbps", bufs=2, space="PSUM")      # [128,256] table psums
        pl_ml = P(name="mlps", bufs=2, space="PSUM")      # [128,320] mlp psums
        pl_sm = P(name="smps", bufs=1, space="PSUM")      # small psums
        sc_p = P(name="scores", bufs=3)
        v_p = P(name="vals8", bufs=3)
        ix_p = P(name="idx", bufs=5)
        tb_p = P(name="tables", bufs=3)
        g_p = P(name="gath", bufs=3)
        h_p = P(name="hid", bufs=3)
        set_p = P(name="sets", bufs=2)
        s18_p = P(name="s18", bufs=2)
        f48_p = P(name="f48", bufs=2)

        # ---------------- global prep: sqG = -0.5*(x0^2+x1^2) [64,256] ----------
        xsq = sc_p.tile([128, 256], F32)
        nc.vector.tensor_tensor(out=xsq[:], in0=xtf[:], in1=xtf[:], op=ALU.mult)
        sq_ps = pl_tb.tile([64, 256], F32, tag="tbps")
        nc.tensor.matmul(sq_ps[:], lhsT=bdgf[:], rhs=xsq[:], start=True, stop=True)
        sqG = wp.tile([64, 256], F32)
        nc.scalar.copy(sqG[:], sq_ps[:])

        def topk_idx(scores_sb, ixt):
            """scores_sb [128,256] f32 (destroyed); returns dense [128,20] i16 of
            ranks 1..20 (rank 0 = self)."""
            for r in range(3):
                v = v_p.tile([128, 8], F32, tag="v8")
                nc.vector.max(v[:], scores_sb[:])
                nc.vector.max_index(ixt[:, 8 * r:8 * r + 8], v[:], scores_sb[:])
                if r < 2:
                    nc.vector.match_replace(scores_sb[:], v[:], scores_sb[:], NEG)
            ixd = ix_p.tile([128, 20], I16, tag="ixd")
            nc.vector.tensor_copy(out=ixd[:], in_=ixt[:, 1:21])
            return ixd

        def reduce_k_max(dst_ap, src_ap_320):
            # src [128, 320] (k-major: m = k*16+pb) -> max over k -> dst [128,16]
            v = src_ap_320.rearrange("p (k pb) -> p pb k", k=20, pb=16)
            nc.vector.tensor_reduce(out=dst_ap, in_=v, axis=AX.X, op=ALU.max)

        for s in range(sets):
            # ---------------- A4/B4 assembly ----------------
            a4 = set_p.tile([4, 2048], F32, tag="a4")
            nc.sync.dma_start(a4[:], a4_d.ap()[:, 2048 * s:2048 * (s + 1)])
            b4 = set_p.tile([4, 2048], F32, tag="b4")
            nc.sync.dma_start(b4[:], b4_d.ap()[:, 2048 * s:2048 * (s + 1)])
            nc.sync.dma_start(b4[2:3, :], sqG[8 * s:8 * s + 8, :])

            x1parts = set_p.tile([128, 256], F32, tag="x1p")   # [(ng,c), (g8,t,pb)]
            x1t8c = set_p.tile([16, 2048], F32, tag="x1c")     # [c, (g8,t,ng,pb)]

            # ---------------- conv1 per graph ----------------
            for g8 in range(8):
                gg = 8 * s + g8
                # kNN scores + selection, per 128-row half
                ixs = []
                for t in range(2):
                    ps = pl_sc.tile([128, 256], F32, tag="scps")
                    nc.tensor.matmul(
                        ps[:], lhsT=a4[:, 256 * g8 + 128 * t:256 * g8 + 128 * (t + 1)],
                        rhs=b4[:, 256 * g8:256 * (g8 + 1)], start=True, stop=True)
                    sc = sc_p.tile([128, 256], F32, tag="sc")
                    nc.scalar.copy(sc[:], ps[:])
                    ixt = ix_p.tile([128, 24], U16, tag="ix")
                    ixs.append(topk_idx(sc, ixt))
                # tables: CTrep / BTrep [128,256]
                pc = pl_tb.tile([128, 256], F32, tag="tbps")
                nc.tensor.matmul(pc[:], lhsT=cwrep[:],
                                 rhs=a4[0:2, 256 * g8:256 * (g8 + 1)],
                                 start=True, stop=True)
                ct = tb_p.tile([128, 256], F32, tag="ct")
                nc.scalar.add(ct[:], pc[:], b0rep[:])
                pb_ = pl_tb.tile([128, 256], F32, tag="tbps")
                nc.tensor.matmul(pb_[:], lhsT=bwrep[:],
                                 rhs=a4[0:2, 256 * g8:256 * (g8 + 1)],
                                 start=True, stop=True)
                bt = tb_p.tile([128, 256], F32, tag="bt")
                nc.scalar.copy(bt[:], pb_[:])
                for t in range(2):
                    cg = g_p.tile([128, 320], F32, tag="cg")
                    nc.gpsimd.ap_gather(cg[:], ct[:], iota[:, 20 * t:20 * t + 20],
                                        channels=128, num_elems=256, d=1, num_idxs=320)
                    bg = g_p.tile([128, 320], F32, tag="bg")
                    nc.gpsimd.ap_gather(bg[:], bt[:], ixs[t][:],
                                        channels=128, num_elems=256, d=1, num_idxs=320)
                    h1 = h_p.tile([128, 320], F32, tag="h1")
                    nc.gpsimd.tensor_tensor(out=h1[:], in0=bg[:], in1=cg[:], op=ALU.add)
                    nc.vector.tensor_scalar_max(out=h1[:], in0=h1[:], scalar1=0.0)
                    m1 = pl_ml.tile([128, 320], F32, tag="mlps")
                    nc.tensor.matmul(m1[:], lhsT=w1bd[:], rhs=h1[:], start=True, stop=True)
                    h2 = h_p.tile([128, 320], F32, tag="h2")
                    nc.scalar.activation(h2[:], m1[:], AF.Relu, bias=b1rep[:])
                    m2 = pl_ml.tile([128, 320], F32, tag="mlps")
                    nc.tensor.matmul(m2[:], lhsT=w2bd[:], rhs=h2[:], start=True, stop=True)
                    reduce_k_max(x1parts[:, 32 * g8 + 16 * t:32 * g8 + 16 * (t + 1)], m2[:])

            # x1 += b2 ; relayout to feature-major
            nc.vector.tensor_scalar_add(out=x1parts[:], in0=x1parts[:], scalar1=b2rep[:])
            x1v = x1t8c[:].rearrange("c (g t ng pb) -> c g t ng pb",
                                     g=8, t=2, ng=8, pb=16)
            for ng in range(8):
                # dst [16c, (g8,t,[ng],pb)] <- src [16c, (g8,t,pb)]
                nc.sync.dma_start(x1v[:, :, :, ng:ng + 1, :],
                                  x1parts[16 * ng:16 * ng + 16, :])

            # sq1 = -0.5 * sum_c x1^2  -> S18B row 16
            x1sq = set_p.tile([16, 2048], F32, tag="x1sq")
            nc.scalar.activation(x1sq[:], x1t8c[:], AF.Square)
            s18a = s18_p.tile([18, 2048], F32, tag="s18a")
            s18b = s18_p.tile([18, 2048], F32, tag="s18b")
            nc.sync.dma_start(s18a[0:16, :], x1t8c[:])
            nc.sync.dma_start(s18a[16:17, :], onesr[:])
            nc.sync.dma_start(s18a[17:18, :], zeror[:])
            nc.sync.dma_start(s18b[0:16, :], x1t8c[:])
            nc.sync.dma_start(s18b[17:18, :], onesr[:])
            sqrow = set_p.tile([1, 2048], F32, tag="sqrow")
            for q in range(4):
                mq = pl_sm.tile([1, 512], F32, tag="smps")
                nc.tensor.matmul(mq[:], lhsT=nh16[:], rhs=x1sq[:, 512 * q:512 * (q + 1)],
                                 start=True, stop=True)
                nc.scalar.copy(sqrow[:, 512 * q:512 * (q + 1)], mq[:])
            nc.sync.dma_start(s18b[16:17, :], sqrow[:])

            f48 = f48_p.tile([48, 2048], F32, tag="f48")
            nc.sync.dma_start(f48[0:16, :], x1t8c[:])

            x2pa = set_p.tile([128, 256], F32, tag="x2pa")
            x2pb = set_p.tile([128, 256], F32, tag="x2pb")

            # ---------------- conv2 per graph ----------------
            for g8 in range(8):
                ix2s = []
                for t in range(2):
                    ps = pl_sc.tile([128, 256], F32, tag="scps")
                    nc.tensor.matmul(
                        ps[:], lhsT=s18a[:, 256 * g8 + 128 * t:256 * g8 + 128 * (t + 1)],
                        rhs=s18b[:, 256 * g8:256 * (g8 + 1)], start=True, stop=True)
                    sc = sc_p.tile([128, 256], F32, tag="sc")
                    nc.scalar.copy(sc[:], ps[:])
                    ixt = ix_p.tile([128, 24], U16, tag="ix")
                    ix2s.append(topk_idx(sc, ixt))
                pq = pl_tb.tile([128, 256], F32, tag="tbps")
                nc.tensor.matmul(pq[:], lhsT=qarep_w[:],
                                 rhs=x1t8c[:, 256 * g8:256 * (g8 + 1)], start=True, stop=True)
                qa = tb_p.tile([128, 256], F32, tag="qa")
                nc.scalar.copy(qa[:], pq[:])
                pq2 = pl_tb.tile([128, 256], F32, tag="tbps")
                nc.tensor.matmul(pq2[:], lhsT=qbrep_w[:],
                                 rhs=x1t8c[:, 256 * g8:256 * (g8 + 1)], start=True, stop=True)
                qb = tb_p.tile([128, 256], F32, tag="qb")
                nc.scalar.copy(qb[:], pq2[:])
                for t in range(2):
                    ga = g_p.tile([128, 320], F32, tag="ga")
                    nc.gpsimd.ap_gather(ga[:], qa[:], ix2s[t][:],
                                        channels=128, num_elems=256, d=1, num_idxs=320)
                    reduce_k_max(x2pa[:, 32 * g8 + 16 * t:32 * g8 + 16 * (t + 1)], ga[:])
                    gb = g_p.tile([128, 320], F32, tag="gb")
                    nc.gpsimd.ap_gather(gb[:], qb[:], ix2s[t][:],
                                        channels=128, num_elems=256, d=1, num_idxs=320)
                    reduce_k_max(x2pb[:, 32 * g8 + 16 * t:32 * g8 + 16 * (t + 1)], gb[:])
                # linear term of conv2 (+bias) -> F48 rows 16..47
                l2 = pl_tb.tile([32, 256], F32, tag="tbps")
                nc.tensor.matmul(l2[:], lhsT=wd2aug[:],
                                 rhs=s18b[:, 256 * g8:256 * (g8 + 1)], start=True, stop=True)
                l2s = tb_p.tile([32, 256], F32, tag="l2s")
                nc.scalar.copy(l2s[:], l2[:])
                nc.sync.dma_start(f48[16:48, 256 * g8:256 * (g8 + 1)], l2s[:])

            # accumulate the aggregated max-terms into F48 rows 16..47
            f48v = f48[:].rearrange("c (g t ng pb) -> c g t ng pb",
                                    g=8, t=2, ng=8, pb=16)
            for ng in range(8):
                nc.gpsimd.dma_start(f48v[16:32, :, :, ng:ng + 1, :],
                                    x2pa[16 * ng:16 * ng + 16, :], accum_op=ALU.add)
                nc.gpsimd.dma_start(f48v[32:48, :, :, ng:ng + 1, :],
                                    x2pb[16 * ng:16 * ng + 16, :], accum_op=ALU.add)

            # ---------------- lin1 + global max pool ----------------
            for g8 in range(8):
                pl = pl_sc.tile([128, 256], F32, tag="scps")
                nc.tensor.matmul(pl[:], lhsT=w1l[:], rhs=f48[:, 256 * g8:256 * (g8 + 1)],
                                 start=True, stop=True)
                nc.vector.tensor_reduce(out=pooledT[:, 8 * s + g8:8 * s + g8 + 1],
                                        in_=pl[:], axis=AX.X, op=ALU.max)

        # ---------------- head MLP ----------------
        nc.vector.tensor_scalar_add(out=pooledT[:], in0=pooledT[:], scalar1=b1l[:])
        hd1 = pl_sm.tile([64, G], F32, tag="smps")
        nc.tensor.matmul(hd1[:], lhsT=mw0[:], rhs=pooledT[:], start=True, stop=True)
        h1s = wp.tile([64, G], F32)
        nc.scalar.activation(h1s[:], hd1[:], AF.Relu, bias=mb0[:])
        hd2 = pl_sm.tile([64, G], F32, tag="smps")
        nc.tensor.matmul(hd2[:], lhsT=mw1[:], rhs=h1s[:], start=True, stop=True)
        h2s = wp.tile([64, G], F32)
        nc.scalar.activation(h2s[:], hd2[:], AF.Relu, bias=mb1[:])
        hd3 = pl_sm.tile([1, G], F32, tag="smps")
        nc.tensor.matmul(hd3[:], lhsT=mw2[:], rhs=h2s[:], start=True, stop=True)
        outs = wp.tile([1, G], F32)
        nc.vector.tensor_scalar_add(out=outs[:], in0=hd3[:], scalar1=mb2[:])
        nc.sync.dma_start(out_d.ap(), outs[:])

    nc.compile()
    return nc


# ---------------------------------------------------------------------------
# Host-side input prep
# ---------------------------------------------------------------------------

def _tile8(w):
    return np.tile(np.asarray(w, np.float32), (1, 8) if w.ndim == 2 else 8)


def _blkdiag8(w):
    w = np.asarray(w, np.float32)
    out = np.zeros((128, 128), np.float32)
    for i in range(8):
        out[16 * i:16 * i + 16, 16 * i:16 * i + 16] = w
    return out


def make_in_maps(inputs):
    x = np.asarray(inputs["x"], np.float32)
    c1_w0 = np.asarray(inputs["c1_w0"], np.float32)
    consts = {}
    cw = c1_w0[:2] - c1_w0[2:4]           # [2,16]
    consts["cwrep"] = np.tile(cw, (1, 8)).astype(np.float32)                # [2,128]
    consts["bwrep"] = np.tile(c1_w0[2:4], (1, 8)).astype(np.float32)        # [2,128]
    consts["b0rep"] = np.tile(np.asarray(inputs["c1_b0"], np.float32), 8)[:, None]
    consts["w1bd"] = _blkdiag8(inputs["c1_w1"])
    consts["b1rep"] = np.tile(np.asarray(inputs["c1_b1"], np.float32), 8)[:, None]
    consts["w2bd"] = _blkdiag8(inputs["c1_w2"])
    consts["b2rep"] = np.tile(np.asarray(inputs["c1_b2"], np.float32), 8)[:, None]
    bdgf = np.zeros((128, 64), np.float32)
    for g in range(64):
        bdgf[2 * g, g] = -0.5
        bdgf[2 * g + 1, g] = -0.5
    consts["bdgf"] = bdgf
    consts["neghalf16"] = np.full((16, 1), -0.5, np.float32)
    consts["onesrow"] = np.ones((1, 2048), np.float32)
    consts["zerorow"] = np.zeros((1, 2048), np.float32)
    iota = np.zeros((128, 40), np.int16)
    for t in range(2):
        for p in range(128):
            iota[p, 20 * t:20 * t + 20] = 128 * t + p
    consts["iotaidx"] = iota
    c2_w0 = np.asarray(inputs["c2_w0"], np.float32)
    consts["wb2repA"] = np.tile(c2_w0[16:32, 0:16], (1, 8)).astype(np.float32)
    consts["wb2repB"] = np.tile(c2_w0[16:32, 16:32], (1, 8)).astype(np.float32)
    wd2aug = np.zeros((18, 32), np.float32)
    wd2aug[0:16] = c2_w0[0:16] - c2_w0[16:32]
    wd2aug[17] = np.asarray(inputs["c2_b0"], np.float32)
    consts["wd2aug"] = wd2aug
    consts["w1l"] = np.asarray(inputs["lin1_w"], np.float32)
    consts["b1l"] = np.asarray(inputs["lin1_b"], np.float32)[:, None]
    consts["mw0"] = np.asarray(inputs["m_w0"], np.float32)
    consts["mb0"] = np.asarray(inputs["m_b0"], np.float32)[:, None]
    consts["mw1"] = np.asarray(inputs["m_w1"], np.float32)
    consts["mb1"] = np.asarray(inputs["m_b1"], np.float32)[:, None]
    consts["mw2"] = np.asarray(inputs["m_w2"], np.float32)
    consts["mb2"] = np.asarray(inputs["m_b2"], np.float32)[:, None]

    in_maps = []
    npc = N * GPC  # nodes per core
    for c in range(NCORES):
        xc = x[c * npc:(c + 1) * npc]                       # [16384, 2]
        xg = xc.reshape(GPC, N, 2)
        m = dict(consts)
        m["xtf"] = xg.transpose(0, 2, 1).reshape(128, 256).copy()   # [2g+f, j]
        rows = xc.T.reshape(2, -1)                          # [f, 256g+j]
        a4 = np.zeros((4, 16384), np.float32)
        a4[0:2] = rows
        a4[2] = 1.0
        m["a4all"] = a4
        b4 = np.zeros((4, 16384), np.float32)
        b4[0:2] = rows
        b4[3] = 1.0
        m["b4all"] = b4
        in_maps.append(m)
    return in_maps


_CACHED = {}


def _get_program(sets=8):
    if sets not in _CACHED:
        _CACHED[sets] = build_program(sets)
    return _CACHED[sets]


def run(inputs, trace=False, **kw):
    nc = _get_program(8)
    in_maps = make_in_maps(inputs)
    res = run_bass_kernel_spmd(nc, in_maps, list(range(NCORES)), trace=trace, **kw)
    out = np.concatenate([res.results[c]["out"].reshape(GPC) for c in range(NCORES)])
    return out.reshape(B, 1).astype(np.float32), res


def kernel(**inputs) -> np.ndarray:
    out, _ = run(inputs, trace=False)
    return out



# revision 7
# speedup vs baseline: 1.0721x; 1.0721x over previous
"""DGCNN-style GNN (2x dynamic-kNN EdgeConv + global pool + MLP head) on 8 Trainium2
NeuronCores, data-parallel over the 512 graphs (64 graphs per core).

Self-contained: hardcodes all shapes; host side only does layout prep (transpose /
tiling / block-diag packing of weights) and sharding.

v2 structure notes (vs the original baseline):
- gpsimd runs ONLY ap_gather, so its ucode library is loaded once (the baseline
  alternated ap_gather / tensor_tensor / SWDGE-dma and paid a ~6us IRAM reload on
  every switch).
- conv2's linear (center+bias) term is folded into lin1's weights on the host
  (lin1 is linear), so the f48 assembly + accumulate-DMAs disappear; lin1 runs as
  PSUM-accumulated matmuls over 512-column chunks.
- top-k keeps all 24 extracted ranks in one uint16 tile and the neighbor gathers
  read ranks 0..23 directly (bitcast to int16); the max-over-k reduce slices ranks
  1..20, so the u16->i16 densifying CAST per topk is gone.
- the EdgeConv center term is fetched with a 16-element gather and broadcast along
  k by a zero-stride AP in the vector add.
- score inputs (x rows, ones, -|x|^2/2) are fully host-precomputed.
"""

import sys

for _p in ("/opt/trn_rl_repo",):
    if _p not in sys.path:
        sys.path.append(_p)

from contextlib import ExitStack

import numpy as np

import concourse.bass as bass
import concourse.tile as tile
from concourse import bacc, mybir
from concourse.bass_utils import run_bass_kernel_spmd

F32 = mybir.dt.float32
U16 = mybir.dt.uint16
I16 = mybir.dt.int16
AF = mybir.ActivationFunctionType
ALU = mybir.AluOpType
AX = mybir.AxisListType

B, N, K = 512, 256, 20
NCORES = 8
GPC = B // NCORES  # graphs per core = 64
NEG = -1.0e30


# ---------------------------------------------------------------------------
# Bass program builder (identical program for every core; all data via inputs)
# ---------------------------------------------------------------------------

def build_program(sets: int = 8):
    G = 8 * sets
    nc = bacc.Bacc("TRN2", target_bir_lowering=False, debug=False)

    def din(name, shape, dtype=F32):
        return nc.declare_dram_parameter(name, list(shape), dtype, isOutput=False)

    # -------------------- DRAM parameters --------------------
    a3_d = din("a3all", [3, 16384])         # rows: x0, x1, ones
    b3_d = din("b3all", [3, 16384])         # rows: x0, x1, -(x0^2+x1^2)/2
    cw3_d = din("cw3", [3, 128])            # [tile8(c1_w0[:2]-c1_w0[2:4]); tile8(c1_b0)]
    bw2_d = din("bw2", [2, 128])            # tile8(c1_w0[2:4])
    w1bd_d = din("w1bd", [128, 128])        # blkdiag8(c1_w1)
    b1rep_d = din("b1rep", [128, 1])
    w2bd_d = din("w2bd", [128, 128])        # blkdiag8(c1_w2)
    b2rep_d = din("b2rep", [128, 1])
    qarep_d = din("wb2repA", [16, 128])     # tile8(c2_w0[16:32, :16])
    qbrep_d = din("wb2repB", [16, 128])     # tile8(c2_w0[16:32, 16:])
    nh16_d = din("neghalf16", [16, 1])      # -0.5
    WA_d = din("WA", [17, 128])             # composed lin1 weight for [x1; 1]
    wla_d = din("wla", [16, 128])           # lin1_w[16:32]
    wlb_d = din("wlb", [16, 128])           # lin1_w[32:48]
    ones_d = din("onesrow", [1, 2048])
    iota16_d = din("iota16", [128, 2], I16)  # [:, t] = 128t + p
    mw0_d = din("mw0", [128, 64])
    mb0_d = din("mb0", [64, 1])
    mw1_d = din("mw1", [64, 64])
    mb1_d = din("mb1", [64, 1])
    mw2_d = din("mw2", [64, 1])
    mb2_d = din("mb2", [1, 1])
    out_d = nc.declare_dram_parameter("out", [1, G], F32, isOutput=True)

    with tile.TileContext(nc) as tc, ExitStack() as ctx:
        P = lambda **kw: ctx.enter_context(tc.tile_pool(**kw))
        wp = P(name="weights", bufs=1)

        def load(dram, shape, dtype=F32):
            t = wp.tile(list(shape), dtype, tag=dram.name)
            nc.sync.dma_start(t[:], dram.ap())
            return t

        cw3 = load(cw3_d, [3, 128])
        bw2 = load(bw2_d, [2, 128])
        w1bd = load(w1bd_d, [128, 128])
        b1rep = load(b1rep_d, [128, 1])
        w2bd = load(w2bd_d, [128, 128])
        b2rep = load(b2rep_d, [128, 1])
        qarep = load(qarep_d, [16, 128])
        qbrep = load(qbrep_d, [16, 128])
        nh16 = load(nh16_d, [16, 1])
        WA = load(WA_d, [17, 128])
        wla = load(wla_d, [16, 128])
        wlb = load(wlb_d, [16, 128])
        onesr = load(ones_d, [1, 2048])
        iota16 = load(iota16_d, [128, 2], I16)
        mw0 = load(mw0_d, [128, 64])
        mb0 = load(mb0_d, [64, 1])
        mw1 = load(mw1_d, [64, 64])
        mb1 = load(mb1_d, [64, 1])
        mw2 = load(mw2_d, [64, 1])
        mb2 = load(mb2_d, [1, 1])

        # persistent core-level tensors
        pooledT = wp.tile([128, G], F32)

        # pools
        pl_sc = P(name="scps", bufs=2, space="PSUM")      # [128,256] score psums
        pl_tb = P(name="tbps", bufs=2, space="PSUM")      # [128,512] table psums
        pl_ml = P(name="mlps", bufs=2, space="PSUM")      # [128,384] mlp psums
        pl_ln = P(name="lnps", bufs=1, space="PSUM")      # [128,512] lin1 psum
        pl_sm = P(name="smps", bufs=1, space="PSUM")      # small psums
        sc_p = P(name="scores", bufs=4)
        v_p = P(name="vals8", bufs=4)
        ix_p = P(name="idx", bufs=8)
        tb_p = P(name="tables", bufs=4)
        g_p = P(name="gath", bufs=4)
        h_p = P(name="hid", bufs=4)
        set_p = P(name="sets", bufs=2)

        def topk24(scores_sb):
            """scores_sb [128,256] f32 (destroyed); returns [128,24] u16 of
            ranks 0..23 (rank 0 = self)."""
            ixt = ix_p.tile([128, 24], U16, tag="ix")
            for r in range(3):
                v = v_p.tile([128, 8], F32, tag="v8")
                nc.vector.max(v[:], scores_sb[:])
                nc.vector.max_index(ixt[:, 8 * r:8 * r + 8], v[:], scores_sb[:])
                if r < 2:
                    nc.vector.match_replace(scores_sb[:], v[:], scores_sb[:], NEG)
            return ixt

        def reduce_k_max(dst_ap, src_ap_384):
            # src [128, 384] (k-major: m = k*16+pb, k=0..23) -> max over k in
            # [1, 21) -> dst [128,16]
            v = src_ap_384.rearrange("p (k pb) -> p pb k", k=24, pb=16)
            nc.vector.tensor_reduce(out=dst_ap, in_=v[:, :, 1:21], axis=AX.X,
                                    op=ALU.max)

        for s in range(sets):
            a3 = set_p.tile([3, 2048], F32, tag="a3")
            nc.sync.dma_start(a3[:], a3_d.ap()[:, 2048 * s:2048 * (s + 1)])
            b3 = set_p.tile([3, 2048], F32, tag="b3")
            nc.sync.dma_start(b3[:], b3_d.ap()[:, 2048 * s:2048 * (s + 1)])

            A = set_p.tile([17, 2048], F32, tag="A")    # rows: x1 (16), ones
            Bt = set_p.tile([17, 2048], F32, tag="B")   # rows: x1 (16), -|x1|^2/2
            nc.sync.dma_start(A[16:17, :], onesr[:])
            x1parts = set_p.tile([128, 256], F32, tag="x1p")   # [(ng,c), (g8,t,pb)]
            x2pa = set_p.tile([128, 256], F32, tag="x2pa")
            x2pb = set_p.tile([128, 256], F32, tag="x2pb")
            x2af = set_p.tile([16, 2048], F32, tag="x2af")
            x2bf = set_p.tile([16, 2048], F32, tag="x2bf")

            # ---------------- conv1 per graph ----------------
            for g8 in range(8):
                c0, c1 = 256 * g8, 256 * (g8 + 1)
                ixs = []
                for t in range(2):
                    ps = pl_sc.tile([128, 256], F32, tag="scps")
                    nc.tensor.matmul(
                        ps[:], lhsT=a3[:, c0 + 128 * t:c0 + 128 * (t + 1)],
                        rhs=b3[:, c0:c1], start=True, stop=True)
                    sc = sc_p.tile([128, 256], F32, tag="sc")
                    nc.scalar.copy(sc[:], ps[:])
                    ixs.append(topk24(sc))
                ptb = pl_tb.tile([128, 512], F32, tag="tbps")
                nc.tensor.matmul(ptb[:, 0:256], lhsT=cw3[:], rhs=a3[:, c0:c1],
                                 start=True, stop=True)
                nc.tensor.matmul(ptb[:, 256:512], lhsT=bw2[:], rhs=a3[0:2, c0:c1],
                                 start=True, stop=True)
                tbl = tb_p.tile([128, 512], F32, tag="cbt")
                nc.scalar.copy(tbl[:], ptb[:])
                ct, bt = tbl[:, 0:256], tbl[:, 256:512]
                for t in range(2):
                    ctv = g_p.tile([128, 16], F32, tag="ctv")
                    nc.gpsimd.ap_gather(ctv[:], ct, iota16[:, t:t + 1],
                                        channels=128, num_elems=256, d=1,
                                        num_idxs=16)
                    bg = g_p.tile([128, 384], F32, tag="bg")
                    nc.gpsimd.ap_gather(bg[:], bt, ixs[t][:].bitcast(I16),
                                        channels=128, num_elems=256, d=1,
                                        num_idxs=384)
                    h1 = h_p.tile([128, 384], F32, tag="h1")
                    nc.vector.tensor_tensor(
                        out=h1[:].rearrange("p (k pb) -> p k pb", k=24, pb=16),
                        in0=bg[:].rearrange("p (k pb) -> p k pb", k=24, pb=16),
                        in1=ctv[:, None, :].broadcast_to((128, 24, 16)),
                        op=ALU.add)
                    h1r = h_p.tile([128, 384], F32, tag="h1r")
                    nc.scalar.activation(h1r[:], h1[:], AF.Relu)
                    m1 = pl_ml.tile([128, 384], F32, tag="mlps")
                    nc.tensor.matmul(m1[:], lhsT=w1bd[:], rhs=h1r[:],
                                     start=True, stop=True)
                    h2 = h_p.tile([128, 384], F32, tag="h2")
                    nc.scalar.activation(h2[:], m1[:], AF.Relu, bias=b1rep[:])
                    m2 = pl_ml.tile([128, 384], F32, tag="mlps")
                    nc.tensor.matmul(m2[:], lhsT=w2bd[:], rhs=h2[:],
                                     start=True, stop=True)
                    reduce_k_max(x1parts[:, 32 * g8 + 16 * t:32 * g8 + 16 * (t + 1)],
                                 m2[:])

            # x1 += b2 ; relayout to feature-major rows of A
            nc.vector.tensor_scalar_add(out=x1parts[:], in0=x1parts[:],
                                        scalar1=b2rep[:])
            Av = A[0:16, :].rearrange("c (g t ng pb) -> c g t ng pb",
                                      g=8, t=2, ng=8, pb=16)
            for ng in range(8):
                nc.sync.dma_start(Av[:, :, :, ng:ng + 1, :],
                                  x1parts[16 * ng:16 * ng + 16, :])
            nc.sync.dma_start(Bt[0:16, :], A[0:16, :])

            # Bt row 16 = -|x1|^2/2
            x1sq = set_p.tile([16, 2048], F32, tag="x1sq")
            nc.scalar.activation(x1sq[:], A[0:16, :], AF.Square)
            sqrow = set_p.tile([1, 2048], F32, tag="sqrow")
            for q in range(4):
                mq = pl_sm.tile([1, 512], F32, tag="smps")
                nc.tensor.matmul(mq[:], lhsT=nh16[:],
                                 rhs=x1sq[:, 512 * q:512 * (q + 1)],
                                 start=True, stop=True)
                nc.scalar.copy(sqrow[:, 512 * q:512 * (q + 1)], mq[:])
            nc.sync.dma_start(Bt[16:17, :], sqrow[:])

            # ---------------- conv2 per graph ----------------
            for g8 in range(8):
                c0, c1 = 256 * g8, 256 * (g8 + 1)
                ix2s = []
                for t in range(2):
                    ps = pl_sc.tile([128, 256], F32, tag="scps")
                    nc.tensor.matmul(
                        ps[:], lhsT=A[:, c0 + 128 * t:c0 + 128 * (t + 1)],
                        rhs=Bt[:, c0:c1], start=True, stop=True)
                    sc = sc_p.tile([128, 256], F32, tag="sc")
                    nc.scalar.copy(sc[:], ps[:])
                    ix2s.append(topk24(sc))
                pq = pl_tb.tile([128, 512], F32, tag="tbps")
                nc.tensor.matmul(pq[:, 0:256], lhsT=qarep[:], rhs=A[0:16, c0:c1],
                                 start=True, stop=True)
                nc.tensor.matmul(pq[:, 256:512], lhsT=qbrep[:], rhs=A[0:16, c0:c1],
                                 start=True, stop=True)
                qtbl = tb_p.tile([128, 512], F32, tag="qab")
                nc.scalar.copy(qtbl[:], pq[:])
                qa, qb = qtbl[:, 0:256], qtbl[:, 256:512]
                for t in range(2):
                    ga = g_p.tile([128, 384], F32, tag="ga")
                    nc.gpsimd.ap_gather(ga[:], qa, ix2s[t][:].bitcast(I16),
                                        channels=128, num_elems=256, d=1,
                                        num_idxs=384)
                    reduce_k_max(x2pa[:, 32 * g8 + 16 * t:32 * g8 + 16 * (t + 1)],
                                 ga[:])
                    gb = g_p.tile([128, 384], F32, tag="gb")
                    nc.gpsimd.ap_gather(gb[:], qb, ix2s[t][:].bitcast(I16),
                                        channels=128, num_elems=256, d=1,
                                        num_idxs=384)
                    reduce_k_max(x2pb[:, 32 * g8 + 16 * t:32 * g8 + 16 * (t + 1)],
                                 gb[:])

            # relayout conv2 max-parts to feature-major
            xav = x2af[:].rearrange("c (g t ng pb) -> c g t ng pb",
                                    g=8, t=2, ng=8, pb=16)
            xbv = x2bf[:].rearrange("c (g t ng pb) -> c g t ng pb",
                                    g=8, t=2, ng=8, pb=16)
            for ng in range(8):
                nc.sync.dma_start(xav[:, :, :, ng:ng + 1, :],
                                  x2pa[16 * ng:16 * ng + 16, :])
                nc.sync.dma_start(xbv[:, :, :, ng:ng + 1, :],
                                  x2pb[16 * ng:16 * ng + 16, :])

            # ---------------- lin1 (accumulated) + global max pool ----------
            for q in range(4):
                pl = pl_ln.tile([128, 512], F32, tag="lnps")
                nc.tensor.matmul(pl[:], lhsT=WA[:],
                                 rhs=A[:, 512 * q:512 * (q + 1)],
                                 start=True, stop=False)
                nc.tensor.matmul(pl[:], lhsT=wla[:],
                                 rhs=x2af[:, 512 * q:512 * (q + 1)],
                                 start=False, stop=False)
                nc.tensor.matmul(pl[:], lhsT=wlb[:],
                                 rhs=x2bf[:, 512 * q:512 * (q + 1)],
                                 start=False, stop=True)
                for h in range(2):
                    g8 = 2 * q + h
                    nc.vector.tensor_reduce(
                        out=pooledT[:, 8 * s + g8:8 * s + g8 + 1],
                        in_=pl[:, 256 * h:256 * (h + 1)], axis=AX.X, op=ALU.max)

        # ---------------- head MLP ----------------
        hd1 = pl_sm.tile([64, G], F32, tag="smps")
        nc.tensor.matmul(hd1[:], lhsT=mw0[:], rhs=pooledT[:], start=True, stop=True)
        h1s = wp.tile([64, G], F32)
        nc.scalar.activation(h1s[:], hd1[:], AF.Relu, bias=mb0[:])
        hd2 = pl_sm.tile([64, G], F32, tag="smps")
        nc.tensor.matmul(hd2[:], lhsT=mw1[:], rhs=h1s[:], start=True, stop=True)
        h2s = wp.tile([64, G], F32)
        nc.scalar.activation(h2s[:], hd2[:], AF.Relu, bias=mb1[:])
        hd3 = pl_sm.tile([1, G], F32, tag="smps")
        nc.tensor.matmul(hd3[:], lhsT=mw2[:], rhs=h2s[:], start=True, stop=True)
        outs = wp.tile([1, G], F32)
        nc.vector.tensor_scalar_add(out=outs[:], in0=hd3[:], scalar1=mb2[:])
        nc.sync.dma_start(out_d.ap(), outs[:])

    nc.compile()
    return nc


# ---------------------------------------------------------------------------
# Host-side input prep
# ---------------------------------------------------------------------------

def _tile8(w):
    return np.tile(np.asarray(w, np.float32), (1, 8) if w.ndim == 2 else 8)


def _blkdiag8(w):
    w = np.asarray(w, np.float32)
    out = np.zeros((128, 128), np.float32)
    for i in range(8):
        out[16 * i:16 * i + 16, 16 * i:16 * i + 16] = w
    return out


def make_in_maps(inputs):
    x = np.asarray(inputs["x"], np.float32)
    c1_w0 = np.asarray(inputs["c1_w0"], np.float32)
    consts = {}
    cw3 = np.zeros((3, 128), np.float32)
    cw3[0:2] = np.tile(c1_w0[:2] - c1_w0[2:4], (1, 8))
    cw3[2] = np.tile(np.asarray(inputs["c1_b0"], np.float32), 8)
    consts["cw3"] = cw3
    consts["bw2"] = np.tile(c1_w0[2:4], (1, 8)).astype(np.float32)
    consts["w1bd"] = _blkdiag8(inputs["c1_w1"])
    consts["b1rep"] = np.tile(np.asarray(inputs["c1_b1"], np.float32), 8)[:, None]
    consts["w2bd"] = _blkdiag8(inputs["c1_w2"])
    consts["b2rep"] = np.tile(np.asarray(inputs["c1_b2"], np.float32), 8)[:, None]
    c2_w0 = np.asarray(inputs["c2_w0"], np.float32)
    consts["wb2repA"] = np.tile(c2_w0[16:32, 0:16], (1, 8)).astype(np.float32)
    consts["wb2repB"] = np.tile(c2_w0[16:32, 16:32], (1, 8)).astype(np.float32)
    consts["neghalf16"] = np.full((16, 1), -0.5, np.float32)
    # lin1 composed with conv2's linear term (lin1 is linear in its input):
    # out = [x1; x2] @ w1l,  x2 = (Wa - Wb)^T-style center term + bias + max-part
    w1l = np.asarray(inputs["lin1_w"], np.float64)     # [48, 128]
    b1l = np.asarray(inputs["lin1_b"], np.float64)     # [128]
    c2b = np.asarray(inputs["c2_b0"], np.float64)      # [32]
    wdiff = np.asarray(c2_w0[0:16], np.float64) - np.asarray(c2_w0[16:32], np.float64)
    WA = np.zeros((17, 128), np.float64)
    WA[0:16] = w1l[0:16] + wdiff @ w1l[16:48]
    WA[16] = c2b @ w1l[16:48] + b1l
    consts["WA"] = WA.astype(np.float32)
    consts["wla"] = np.asarray(w1l[16:32], np.float32)
    consts["wlb"] = np.asarray(w1l[32:48], np.float32)
    consts["onesrow"] = np.ones((1, 2048), np.float32)
    iota16 = np.zeros((128, 2), np.int16)
    for t in range(2):
        iota16[:, t] = 128 * t + np.arange(128)
    consts["iota16"] = iota16
    consts["mw0"] = np.asarray(inputs["m_w0"], np.float32)
    consts["mb0"] = np.asarray(inputs["m_b0"], np.float32)[:, None]
    consts["mw1"] = np.asarray(inputs["m_w1"], np.float32)
    consts["mb1"] = np.asarray(inputs["m_b1"], np.float32)[:, None]
    consts["mw2"] = np.asarray(inputs["m_w2"], np.float32)
    consts["mb2"] = np.asarray(inputs["m_b2"], np.float32)[:, None]

    in_maps = []
    npc = N * GPC  # nodes per core
    for c in range(NCORES):
        xc = x[c * npc:(c + 1) * npc]                       # [16384, 2]
        rows = xc.T.reshape(2, -1)                          # [f, 256g+j]
        m = dict(consts)
        a3 = np.empty((3, 16384), np.float32)
        a3[0:2] = rows
        a3[2] = 1.0
        m["a3all"] = a3
        b3 = np.empty((3, 16384), np.float32)
        b3[0:2] = rows
        b3[2] = -0.5 * (rows[0] ** 2 + rows[1] ** 2)
        m["b3all"] = b3
        in_maps.append(m)
    return in_maps


_CACHED = {}


def _get_program(sets=8):
    if sets not in _CACHED:
        _CACHED[sets] = build_program(sets)
    return _CACHED[sets]


def run(inputs, trace=False, **kw):
    nc = _get_program(8)
    in_maps = make_in_maps(inputs)
    res = run_bass_kernel_spmd(nc, in_maps, list(range(NCORES)), trace=trace, **kw)
    out = np.concatenate([res.results[c]["out"].reshape(GPC) for c in range(NCORES)])
    return out.reshape(B, 1).astype(np.float32), res


def kernel(**inputs) -> np.ndarray:
    out, _ = run(inputs, trace=False)
    return out


# revision 9
# speedup vs baseline: 1.8042x; 1.6828x over previous
"""DGCNN-style GNN (2x dynamic-kNN EdgeConv + global pool + MLP head) on 8 Trainium2
NeuronCores, data-parallel over the 512 graphs (64 graphs per core).

v3a structure notes:
- gpsimd runs ONLY ap_gather (one ucode library, loaded once).
- conv1 gathers a d=2-interleaved table (bt, ct) with ONE gather per (graph,
  half): ranks 0..20; rank 0 is always self (max score), so the gathered ct
  component at k=0 IS the center term — broadcast along k in the vector add.
  This kills the separate iota/center gather.
- conv2 gathers a d=2-interleaved table (qa, qb) with ONE gather per (graph,
  half) over ranks 1..20.
- conv2's linear (center+bias) term is folded into lin1's weights on the host;
  lin1 runs as PSUM-accumulated matmuls over 512-column chunks.
- emission order is software-pipelined per set (scores/topk of graph g, gathers
  of g-1, MLP/reduce of g-2) so DVE/PE/ACT work hides behind the Q7 gathers.
"""

import sys

for _p in ("/opt/trn_rl_repo",):
    if _p not in sys.path:
        sys.path.append(_p)

from contextlib import ExitStack

import numpy as np

import concourse.bass as bass
import concourse.tile as tile
from concourse import bacc, mybir
from concourse.bass_utils import run_bass_kernel_spmd

F32 = mybir.dt.float32
U16 = mybir.dt.uint16
I16 = mybir.dt.int16
AF = mybir.ActivationFunctionType
ALU = mybir.AluOpType
AX = mybir.AxisListType

B, N, K = 512, 256, 20
NCORES = 8
GPC = B // NCORES  # graphs per core = 64
NEG = -1.0e30


def build_program(sets: int = 8):
    G = 8 * sets
    nc = bacc.Bacc("TRN2", target_bir_lowering=False, debug=False)

    def din(name, shape, dtype=F32):
        return nc.declare_dram_parameter(name, list(shape), dtype, isOutput=False)

    a3_d = din("a3all", [3, 16384])         # rows: x0, x1, ones
    b3_d = din("b3all", [3, 16384])         # rows: x0, x1, -(x0^2+x1^2)/2
    cw3_d = din("cw3", [3, 128])
    bw2_d = din("bw2", [2, 128])
    w1bd_d = din("w1bd", [128, 128])
    b1rep_d = din("b1rep", [128, 1])
    w2bd_d = din("w2bd", [128, 128])
    b2rep_d = din("b2rep", [128, 1])
    qarep_d = din("wb2repA", [16, 128])
    qbrep_d = din("wb2repB", [16, 128])
    nh16_d = din("neghalf16", [16, 1])
    WA_d = din("WA", [17, 128])
    wla_d = din("wla", [16, 128])
    wlb_d = din("wlb", [16, 128])
    ones_d = din("onesrow", [1, 2048])
    mw0_d = din("mw0", [128, 64])
    mb0_d = din("mb0", [64, 1])
    mw1_d = din("mw1", [64, 64])
    mb1_d = din("mb1", [64, 1])
    mw2_d = din("mw2", [64, 1])
    mb2_d = din("mb2", [1, 1])
    out_d = nc.declare_dram_parameter("out", [1, G], F32, isOutput=True)

    with tile.TileContext(nc) as tc, ExitStack() as ctx:
        P = lambda **kw: ctx.enter_context(tc.tile_pool(**kw))
        wp = P(name="weights", bufs=1)

        def load(dram, shape, dtype=F32):
            t = wp.tile(list(shape), dtype, tag=dram.name)
            nc.sync.dma_start(t[:], dram.ap())
            return t

        cw3 = load(cw3_d, [3, 128])
        bw2 = load(bw2_d, [2, 128])
        w1bd = load(w1bd_d, [128, 128])
        b1rep = load(b1rep_d, [128, 1])
        w2bd = load(w2bd_d, [128, 128])
        b2rep = load(b2rep_d, [128, 1])
        qarep = load(qarep_d, [16, 128])
        qbrep = load(qbrep_d, [16, 128])
        nh16 = load(nh16_d, [16, 1])
        WA = load(WA_d, [17, 128])
        wla = load(wla_d, [16, 128])
        wlb = load(wlb_d, [16, 128])
        onesr = load(ones_d, [1, 2048])
        mw0 = load(mw0_d, [128, 64])
        mb0 = load(mb0_d, [64, 1])
        mw1 = load(mw1_d, [64, 64])
        mb1 = load(mb1_d, [64, 1])
        mw2 = load(mw2_d, [64, 1])
        mb2 = load(mb2_d, [1, 1])

        pooledT = wp.tile([128, G], F32)

        pl_sc = P(name="scps", bufs=2, space="PSUM")      # [128,256]
        pl_tb = P(name="tbps", bufs=2, space="PSUM")      # [128,512]
        pl_ml = P(name="mlps", bufs=2, space="PSUM")      # [128,320]
        pl_ln = P(name="lnps", bufs=1, space="PSUM")      # [128,512]
        pl_sm = P(name="smps", bufs=1, space="PSUM")
        sc_p = P(name="scores", bufs=4)
        v_p = P(name="vals8", bufs=4)
        ix_p = P(name="idx", bufs=8)
        tb_p = P(name="tables", bufs=4)
        g_p = P(name="gath", bufs=4)
        h_p = P(name="hid", bufs=4)
        set_p = P(name="sets", bufs=2)

        def topk24(scores_sb):
            """scores_sb [128,256] f32 (destroyed) -> [128,24] u16 ranks 0..23."""
            ixt = ix_p.tile([128, 24], U16, tag="ix")
            for r in range(3):
                v = v_p.tile([128, 8], F32, tag="v8")
                nc.vector.max(v[:], scores_sb[:])
                nc.vector.max_index(ixt[:, 8 * r:8 * r + 8], v[:], scores_sb[:])
                if r < 2:
                    nc.vector.match_replace(scores_sb[:], v[:], scores_sb[:], NEG)
            return ixt

        for s in range(sets):
            a3 = set_p.tile([3, 2048], F32, tag="a3")
            nc.sync.dma_start(a3[:], a3_d.ap()[:, 2048 * s:2048 * (s + 1)])
            b3 = set_p.tile([3, 2048], F32, tag="b3")
            nc.sync.dma_start(b3[:], b3_d.ap()[:, 2048 * s:2048 * (s + 1)])

            A = set_p.tile([17, 2048], F32, tag="A")    # rows: x1 (16), ones
            Bt = set_p.tile([17, 2048], F32, tag="B")   # rows: x1 (16), -|x1|^2/2
            nc.sync.dma_start(A[16:17, :], onesr[:])
            x1parts = set_p.tile([128, 256], F32, tag="x1p")
            x2pa = set_p.tile([128, 256], F32, tag="x2pa")
            x2pb = set_p.tile([128, 256], F32, tag="x2pb")
            x2af = set_p.tile([16, 2048], F32, tag="x2af")
            x2bf = set_p.tile([16, 2048], F32, tag="x2bf")

            # ---------------- conv1, software-pipelined over graphs ---------
            ixs1 = [None] * 8      # [g8] -> (ixt_t0, ixt_t1)
            tbl1 = [None] * 8      # [g8] -> table tile
            g1s = [None] * 8       # [g8] -> (G1_t0, G1_t1)

            def c1_scores_topk(g8):
                c0, c1 = 256 * g8, 256 * (g8 + 1)
                pair = []
                for t in range(2):
                    ps = pl_sc.tile([128, 256], F32, tag="scps")
                    nc.tensor.matmul(
                        ps[:], lhsT=a3[:, c0 + 128 * t:c0 + 128 * (t + 1)],
                        rhs=b3[:, c0:c1], start=True, stop=True)
                    sc = sc_p.tile([128, 256], F32, tag="sc")
                    nc.scalar.copy(sc[:], ps[:])
                    pair.append(topk24(sc))
                ixs1[g8] = pair
                # tables: psum cols 0:256 = bt (comp0), 256:512 = ct (comp1)
                ptb = pl_tb.tile([128, 512], F32, tag="tbps")
                nc.tensor.matmul(ptb[:, 0:256], lhsT=bw2[:], rhs=a3[0:2, c0:c1],
                                 start=True, stop=True)
                nc.tensor.matmul(ptb[:, 256:512], lhsT=cw3[:], rhs=a3[:, c0:c1],
                                 start=True, stop=True)
                tbl = tb_p.tile([128, 512], F32, tag="cbt")
                nc.scalar.copy(
                    tbl[:].rearrange("p (n d) -> p n d", n=256, d=2),
                    ptb[:].rearrange("p (d n) -> p n d", d=2, n=256))
                tbl1[g8] = tbl

            def c1_gather(g8):
                pair = []
                t2 = tbl1[g8][:].rearrange("p (n d) -> p n d", n=256, d=2)
                for t in range(2):
                    g1 = g_p.tile([128, 672], F32, tag="g1")
                    nc.gpsimd.ap_gather(g1[:], t2,
                                        ixs1[g8][t][:, 0:21].bitcast(I16),
                                        channels=128, num_elems=256, d=2,
                                        num_idxs=336)
                    pair.append(g1)
                g1s[g8] = pair

            def c1_mlp(g8):
                for t in range(2):
                    g1 = g1s[g8][t]
                    gv = g1[:].rearrange("p (k pb d) -> p k pb d",
                                         k=21, pb=16, d=2)
                    h1 = h_p.tile([128, 320], F32, tag="h1")
                    nc.vector.tensor_tensor(
                        out=h1[:].rearrange("p (k pb) -> p k pb", k=20, pb=16),
                        in0=gv[:, 1:21, :, 0],
                        in1=gv[:, 0:1, :, 1].broadcast_to((128, 20, 16)),
                        op=ALU.add)
                    h1r = h_p.tile([128, 320], F32, tag="h1r")
                    nc.scalar.activation(h1r[:], h1[:], AF.Relu)
                    m1 = pl_ml.tile([128, 320], F32, tag="mlps")
                    nc.tensor.matmul(m1[:], lhsT=w1bd[:], rhs=h1r[:],
                                     start=True, stop=True)
                    h2 = h_p.tile([128, 320], F32, tag="h2")
                    nc.scalar.activation(h2[:], m1[:], AF.Relu, bias=b1rep[:])
                    m2 = pl_ml.tile([128, 320], F32, tag="mlps")
                    nc.tensor.matmul(m2[:], lhsT=w2bd[:], rhs=h2[:],
                                     start=True, stop=True)
                    nc.vector.tensor_reduce(
                        out=x1parts[:, 32 * g8 + 16 * t:32 * g8 + 16 * (t + 1)],
                        in_=m2[:].rearrange("p (k pb) -> p pb k", k=20, pb=16),
                        axis=AX.X, op=ALU.max)

            for gg in range(10):
                if gg < 8:
                    c1_scores_topk(gg)
                if 1 <= gg <= 8:
                    c1_gather(gg - 1)
                if gg >= 2:
                    c1_mlp(gg - 2)

            # x1 += b2 ; relayout to feature-major rows of A
            nc.vector.tensor_scalar_add(out=x1parts[:], in0=x1parts[:],
                                        scalar1=b2rep[:])
            Av = A[0:16, :].rearrange("c (g t ng pb) -> c g t ng pb",
                                      g=8, t=2, ng=8, pb=16)
            for ng in range(8):
                nc.sync.dma_start(Av[:, :, :, ng:ng + 1, :],
                                  x1parts[16 * ng:16 * ng + 16, :])
            nc.sync.dma_start(Bt[0:16, :], A[0:16, :])

            # Bt row 16 = -|x1|^2/2
            sqrow = set_p.tile([1, 2048], F32, tag="sqrow")
            for q in range(4):
                xsq = h_p.tile([16, 512], F32, tag="xsq")
                nc.scalar.activation(xsq[:], A[0:16, 512 * q:512 * (q + 1)],
                                     AF.Square)
                mq = pl_sm.tile([1, 512], F32, tag="smps")
                nc.tensor.matmul(mq[:], lhsT=nh16[:], rhs=xsq[:],
                                 start=True, stop=True)
                nc.scalar.copy(sqrow[:, 512 * q:512 * (q + 1)], mq[:])
            nc.sync.dma_start(Bt[16:17, :], sqrow[:])

            # ---------------- conv2, software-pipelined over graphs ---------
            ixs2 = [None] * 8
            tbl2 = [None] * 8
            g2s = [None] * 8

            def c2_scores_topk(g8):
                c0, c1 = 256 * g8, 256 * (g8 + 1)
                pair = []
                for t in range(2):
                    ps = pl_sc.tile([128, 256], F32, tag="scps")
                    nc.tensor.matmul(
                        ps[:], lhsT=A[:, c0 + 128 * t:c0 + 128 * (t + 1)],
                        rhs=Bt[:, c0:c1], start=True, stop=True)
                    sc = sc_p.tile([128, 256], F32, tag="sc")
                    nc.scalar.copy(sc[:], ps[:])
                    pair.append(topk24(sc))
                ixs2[g8] = pair
                pq = pl_tb.tile([128, 512], F32, tag="tbps")
                nc.tensor.matmul(pq[:, 0:256], lhsT=qarep[:], rhs=A[0:16, c0:c1],
                                 start=True, stop=True)
                nc.tensor.matmul(pq[:, 256:512], lhsT=qbrep[:], rhs=A[0:16, c0:c1],
                                 start=True, stop=True)
                qtbl = tb_p.tile([128, 512], F32, tag="qab")
                nc.scalar.copy(
                    qtbl[:].rearrange("p (n d) -> p n d", n=256, d=2),
                    pq[:].rearrange("p (d n) -> p n d", d=2, n=256))
                tbl2[g8] = qtbl

            def c2_gather(g8):
                pair = []
                t2 = tbl2[g8][:].rearrange("p (n d) -> p n d", n=256, d=2)
                for t in range(2):
                    g2 = g_p.tile([128, 672], F32, tag="g2")
                    nc.gpsimd.ap_gather(g2[:], t2,
                                        ixs2[g8][t][:, 0:21].bitcast(I16),
                                        channels=128, num_elems=256, d=2,
                                        num_idxs=336)
                    pair.append(g2)
                g2s[g8] = pair

            def c2_reduce(g8):
                for t in range(2):
                    g2 = g2s[g8][t]
                    gv = g2[:].rearrange("p (k pb d) -> p pb d k",
                                         k=21, pb=16, d=2)
                    nc.vector.tensor_reduce(
                        out=x2pa[:, 32 * g8 + 16 * t:32 * g8 + 16 * (t + 1)],
                        in_=gv[:, :, 0, 1:21], axis=AX.X, op=ALU.max)
                    nc.vector.tensor_reduce(
                        out=x2pb[:, 32 * g8 + 16 * t:32 * g8 + 16 * (t + 1)],
                        in_=gv[:, :, 1, 1:21], axis=AX.X, op=ALU.max)

            for gg in range(10):
                if gg < 8:
                    c2_scores_topk(gg)
                if 1 <= gg <= 8:
                    c2_gather(gg - 1)
                if gg >= 2:
                    c2_reduce(gg - 2)

            # relayout conv2 max-parts to feature-major
            xav = x2af[:].rearrange("c (g t ng pb) -> c g t ng pb",
                                    g=8, t=2, ng=8, pb=16)
            xbv = x2bf[:].rearrange("c (g t ng pb) -> c g t ng pb",
                                    g=8, t=2, ng=8, pb=16)
            for ng in range(8):
                nc.sync.dma_start(xav[:, :, :, ng:ng + 1, :],
                                  x2pa[16 * ng:16 * ng + 16, :])
                nc.sync.dma_start(xbv[:, :, :, ng:ng + 1, :],
                                  x2pb[16 * ng:16 * ng + 16, :])

            # ---------------- lin1 (accumulated) + global max pool ----------
            for q in range(4):
                pl = pl_ln.tile([128, 512], F32, tag="lnps")
                nc.tensor.matmul(pl[:], lhsT=WA[:],
                                 rhs=A[:, 512 * q:512 * (q + 1)],
                                 start=True, stop=False)
                nc.tensor.matmul(pl[:], lhsT=wla[:],
                                 rhs=x2af[:, 512 * q:512 * (q + 1)],
                                 start=False, stop=False)
                nc.tensor.matmul(pl[:], lhsT=wlb[:],
                                 rhs=x2bf[:, 512 * q:512 * (q + 1)],
                                 start=False, stop=True)
                for h in range(2):
                    g8 = 2 * q + h
                    nc.vector.tensor_reduce(
                        out=pooledT[:, 8 * s + g8:8 * s + g8 + 1],
                        in_=pl[:, 256 * h:256 * (h + 1)], axis=AX.X, op=ALU.max)

        # ---------------- head MLP ----------------
        hd1 = pl_sm.tile([64, G], F32, tag="smps")
        nc.tensor.matmul(hd1[:], lhsT=mw0[:], rhs=pooledT[:], start=True, stop=True)
        h1s = wp.tile([64, G], F32)
        nc.scalar.activation(h1s[:], hd1[:], AF.Relu, bias=mb0[:])
        hd2 = pl_sm.tile([64, G], F32, tag="smps")
        nc.tensor.matmul(hd2[:], lhsT=mw1[:], rhs=h1s[:], start=True, stop=True)
        h2s = wp.tile([64, G], F32)
        nc.scalar.activation(h2s[:], hd2[:], AF.Relu, bias=mb1[:])
        hd3 = pl_sm.tile([1, G], F32, tag="smps")
        nc.tensor.matmul(hd3[:], lhsT=mw2[:], rhs=h2s[:], start=True, stop=True)
        outs = wp.tile([1, G], F32)
        nc.vector.tensor_scalar_add(out=outs[:], in0=hd3[:], scalar1=mb2[:])
        nc.sync.dma_start(out_d.ap(), outs[:])

    nc.compile()
    return nc


# ---------------------------------------------------------------------------
# Host-side input prep
# ---------------------------------------------------------------------------

def _blkdiag8(w):
    w = np.asarray(w, np.float32)
    out = np.zeros((128, 128), np.float32)
    for i in range(8):
        out[16 * i:16 * i + 16, 16 * i:16 * i + 16] = w
    return out


def make_in_maps(inputs):
    x = np.asarray(inputs["x"], np.float32)
    c1_w0 = np.asarray(inputs["c1_w0"], np.float32)
    consts = {}
    cw3 = np.zeros((3, 128), np.float32)
    cw3[0:2] = np.tile(c1_w0[:2] - c1_w0[2:4], (1, 8))
    cw3[2] = np.tile(np.asarray(inputs["c1_b0"], np.float32), 8)
    consts["cw3"] = cw3
    consts["bw2"] = np.tile(c1_w0[2:4], (1, 8)).astype(np.float32)
    consts["w1bd"] = _blkdiag8(inputs["c1_w1"])
    consts["b1rep"] = np.tile(np.asarray(inputs["c1_b1"], np.float32), 8)[:, None]
    consts["w2bd"] = _blkdiag8(inputs["c1_w2"])
    consts["b2rep"] = np.tile(np.asarray(inputs["c1_b2"], np.float32), 8)[:, None]
    c2_w0 = np.asarray(inputs["c2_w0"], np.float32)
    consts["wb2repA"] = np.tile(c2_w0[16:32, 0:16], (1, 8)).astype(np.float32)
    consts["wb2repB"] = np.tile(c2_w0[16:32, 16:32], (1, 8)).astype(np.float32)
    consts["neghalf16"] = np.full((16, 1), -0.5, np.float32)
    w1l = np.asarray(inputs["lin1_w"], np.float64)
    b1l = np.asarray(inputs["lin1_b"], np.float64)
    c2b = np.asarray(inputs["c2_b0"], np.float64)
    wdiff = np.asarray(c2_w0[0:16], np.float64) - np.asarray(c2_w0[16:32], np.float64)
    WA = np.zeros((17, 128), np.float64)
    WA[0:16] = w1l[0:16] + wdiff @ w1l[16:48]
    WA[16] = c2b @ w1l[16:48] + b1l
    consts["WA"] = WA.astype(np.float32)
    consts["wla"] = np.asarray(w1l[16:32], np.float32)
    consts["wlb"] = np.asarray(w1l[32:48], np.float32)
    consts["onesrow"] = np.ones((1, 2048), np.float32)
    consts["mw0"] = np.asarray(inputs["m_w0"], np.float32)
    consts["mb0"] = np.asarray(inputs["m_b0"], np.float32)[:, None]
    consts["mw1"] = np.asarray(inputs["m_w1"], np.float32)
    consts["mb1"] = np.asarray(inputs["m_b1"], np.float32)[:, None]
    consts["mw2"] = np.asarray(inputs["m_w2"], np.float32)
    consts["mb2"] = np.asarray(inputs["m_b2"], np.float32)[:, None]

    in_maps = []
    npc = N * GPC
    for c in range(NCORES):
        xc = x[c * npc:(c + 1) * npc]
        rows = xc.T.reshape(2, -1)
        m = dict(consts)
        a3 = np.empty((3, 16384), np.float32)
        a3[0:2] = rows
        a3[2] = 1.0
        m["a3all"] = a3
        b3 = np.empty((3, 16384), np.float32)
        b3[0:2] = rows
        b3[2] = -0.5 * (rows[0] ** 2 + rows[1] ** 2)
        m["b3all"] = b3
        in_maps.append(m)
    return in_maps


_CACHED = {}


def _get_program(sets=8):
    if sets not in _CACHED:
        _CACHED[sets] = build_program(sets)
    return _CACHED[sets]


def run(inputs, trace=False, **kw):
    nc = _get_program(8)
    in_maps = make_in_maps(inputs)
    res = run_bass_kernel_spmd(nc, in_maps, list(range(NCORES)), trace=trace, **kw)
    out = np.concatenate([res.results[c]["out"].reshape(GPC) for c in range(NCORES)])
    return out.reshape(B, 1).astype(np.float32), res


def kernel(**inputs) -> np.ndarray:
    out, _ = run(inputs, trace=False)
    return out
